# revision 17
# baseline (speedup 1.0000x reference)
"""Causal single-head attention (B=16, S=2048, E=1024, H=64) on 8 TRN2 cores.

Sharding: data-parallel over batch, 2 batches per core. Host pre-transposes
x to xT [E, S] per batch (layout prep only; contraction dim must be on SBUF
partitions) and pre-rounds fp32 data to the fp32r grid (11-bit mantissa) so
matmuls run at full PE rate (1 cycle/row) instead of fp32's 4 cycles/row.

Per-core dataflow (per batch):
  pass1: psum = [Wq|Wk].T @ xT  -> qT @ partitions 0:64, (kT @ 64:128 spare)
  pass2: psum = [Wk|Wv].T @ xT  -> kT @ partitions 0:64, vT @ 64:128
  scoresT[j,i] = kT.T @ qT (K=64), exp(s/8) fused on ACT -> attnT (fp32r)
  causal: computed only for j-blocks <= i-chunk; diagonal 128x128 blocks
  masked by a 0/1 upper-tri multiply; PV skips fully-invalid columns.
  PV: outT_aug[65, i] = v_aug.T @ attnT accumulated over j-blocks, where
  v_aug = [v | 1] so row 64 is the softmax denominator.
  normalize: PE-transpose outT_aug 128-col blocks -> [128, 65] PSUM,
  DVE reciprocal of col 64, tensor_scalar multiply -> out [s, h] natural.
"""
import os
import numpy as np
from contextlib import ExitStack

import concourse.bass as bass
import concourse.bacc as bacc
import concourse.tile as tile
import concourse.mybir as mybir
from concourse import bass_utils

B, S, E, H = 16, 2048, 1024, 64
NCORES = 8
BPC = B // NCORES          # batches per core
KC = E // 128              # contraction chunks
NIB = S // 128             # 128-row blocks per sequence
NCH = S // 512             # 512-wide i-chunks

F32 = mybir.dt.float32
F32R = mybir.dt.float32r

LAST_RESULT = None


def _round_fp32r(a: np.ndarray) -> np.ndarray:
    u = np.ascontiguousarray(a, dtype=np.float32).view(np.uint32)
    return (((u.astype(np.uint64) + 0x800) & 0xFFFFF000).astype(np.uint32)).view(np.float32)


def _build():
    nc = bacc.Bacc("TRN2", target_bir_lowering=False, debug=False)
    xt_d = nc.dram_tensor("xt", (BPC, E, S), F32R, kind="ExternalInput").ap()
    wqk_d = nc.dram_tensor("wqk", (E, 128), F32R, kind="ExternalInput").ap()
    wkv_d = nc.dram_tensor("wkv", (E, 128), F32R, kind="ExternalInput").ap()
    ident_d = nc.dram_tensor("ident", (128, 128), F32, kind="ExternalInput").ap()
    idhi_d = nc.dram_tensor("idhi", (128, 64), F32R, kind="ExternalInput").ap()
    tri_d = nc.dram_tensor("tri", (128, 128), F32R, kind="ExternalInput").ap()
    out_d = nc.dram_tensor("out", (BPC, S, H), F32, kind="ExternalOutput").ap()

    with tile.TileContext(nc) as tc, ExitStack() as ctx:
        consts = ctx.enter_context(tc.tile_pool(name="consts", bufs=1))
        xpool = ctx.enter_context(tc.tile_pool(name="xpool", bufs=3))
        qkv = ctx.enter_context(tc.tile_pool(name="qkv", bufs=2))
        vaugp = ctx.enter_context(tc.tile_pool(name="vaug", bufs=2))
        attnp = ctx.enter_context(tc.tile_pool(name="attn", bufs=2))
        outp = ctx.enter_context(tc.tile_pool(name="outp", bufs=2))
        stagep = ctx.enter_context(tc.tile_pool(name="stage", bufs=2))
        recp = ctx.enter_context(tc.tile_pool(name="recp", bufs=4))
        # PSUM: proj 2x2 banks + score 2 banks + pv 1 + tr 1 = 8 banks
        proj_ps = ctx.enter_context(tc.tile_pool(name="proj_ps", bufs=2, space="PSUM"))
        score_ps = ctx.enter_context(tc.tile_pool(name="score_ps", bufs=2, space="PSUM"))
        pv_ps = ctx.enter_context(tc.tile_pool(name="pv_ps", bufs=1, space="PSUM"))
        tr_ps = ctx.enter_context(tc.tile_pool(name="tr_ps", bufs=1, space="PSUM"))

        wqk = consts.tile([128, KC * 128], F32R, tag="wqk")
        wkv = consts.tile([128, KC * 128], F32R, tag="wkv")
        nc.sync.dma_start(wqk[:].rearrange("p (c m) -> p c m", c=KC),
                          wqk_d.rearrange("(c p) m -> p c m", p=128))
        nc.sync.dma_start(wkv[:].rearrange("p (c m) -> p c m", c=KC),
                          wkv_d.rearrange("(c p) m -> p c m", p=128))
        ident = consts.tile([128, 128], F32, tag="ident")
        nc.sync.dma_start(ident[:], ident_d)
        idhi = consts.tile([128, 64], F32R, tag="idhi")
        nc.sync.dma_start(idhi[:], idhi_d)
        tri = consts.tile([128, 128], F32R, tag="tri")
        nc.sync.dma_start(tri[:], tri_d)

        for b in range(BPC):
            # ---- projections: two packed passes, column-group outer so each
            # x half-chunk [128, 1024] is consumed by 4 matmuls then released
            qkT = qkv.tile([128, S], F32R, tag="qkT")   # rows 0:64 qT, 64:128 kT(spare)
            kvT = qkv.tile([128, S], F32R, tag="kvT")   # rows 0:64 kT, 64:128 vT
            for g in range(2):                           # 1024-col groups
                xgs = []
                for h in range(2):
                    xg = xpool.tile([128, 4 * 1024], F32R, tag="xt")
                    xgs.append(xg)
                for seg in range(2):                     # seg-major so the first
                    for h in range(2):                   # [E,512] slab lands early
                        dst = (xgs[h][:].rearrange("p (c s) -> p c s", c=4)
                               [:, :, seg * 512:(seg + 1) * 512])
                        src = (xt_d[b, h * 512:(h + 1) * 512,
                                    g * 1024 + seg * 512: g * 1024 + (seg + 1) * 512]
                               .rearrange("(c p) s -> p c s", p=128))
                        if b == 0 and g == 0 and seg == 0:
                            # split the startup-critical slab per chunk so the
                            # first projection matmuls begin ASAP
                            for c4 in range(4):
                                nc.sync.dma_start(dst[:, c4], src[:, c4])
                        else:
                            nc.sync.dma_start(dst, src)
                for seg in range(2):
                    ps1 = proj_ps.tile([128, 512], F32, tag="proj")
                    ps2 = proj_ps.tile([128, 512], F32, tag="proj")
                    for c in range(KC):
                        base = (c % 4) * 1024 + seg * 512
                        rhs = xgs[c // 4][:, base:base + 512]
                        nc.tensor.matmul(ps1[:], wqk[:, c * 128:(c + 1) * 128], rhs,
                                         start=(c == 0), stop=(c == KC - 1))
                        nc.tensor.matmul(ps2[:], wkv[:, c * 128:(c + 1) * 128], rhs,
                                         start=(c == 0), stop=(c == KC - 1))
                    off = g * 1024 + seg * 512
                    nc.vector.tensor_copy(qkT[:, off:off + 512], ps1[:])
                    nc.vector.tensor_copy(kvT[:, off:off + 512], ps2[:])

            # ---- v natural + ones column (v_aug [128, 16*65])
            v_aug = vaugp.tile([128, NIB * 65], F32R, tag="v_aug")
            nc.vector.memset(v_aug[:].bitcast(F32), 1.0)
            for jb in range(NIB):
                vtr = tr_ps.tile([128, 64], F32R, tag="tr")
                nc.tensor.transpose(vtr[:], kvT[64:128, jb * 128:(jb + 1) * 128], idhi[64:128, :])
                nc.vector.tensor_copy(v_aug[:, jb * 65: jb * 65 + 64], vtr[:])

            outT = outp.tile([65, S], F32, tag="outT")
            stage = stagep.tile([128, NIB * 64], F32, tag="stage")
            for ch in range(NCH):
                njb = 4 * ch + 4
                # ---- scoresT + exp, waves of 2 j-blocks
                attn = attnp.tile([128, njb * 512], F32R, tag="attn")
                for w0 in range(0, njb, 2):
                    nw = min(2, njb - w0)
                    sps = score_ps.tile([128, 1024], F32, tag="score")
                    for q in range(nw):
                        jb = w0 + q
                        nc.tensor.matmul(
                            sps[:, q * 512:(q + 1) * 512],
                            kvT[0:64, jb * 128:(jb + 1) * 128],
                            qkT[0:64, ch * 512:(ch + 1) * 512],
                            start=True, stop=True)
                    nc.scalar.activation(attn[:, w0 * 512:(w0 + nw) * 512],
                                         sps[:, 0:nw * 512],
                                         mybir.ActivationFunctionType.Exp, scale=0.125)
                # ---- causal mask on the 4 diagonal 128x128 blocks
                for k in range(4):
                    jb = 4 * ch + k
                    blk = attn[:, jb * 512 + k * 128: jb * 512 + (k + 1) * 128]
                    nc.vector.tensor_mul(blk, blk, tri[:])
                # ---- PV accumulate over j-blocks
                pso = pv_ps.tile([65, 512], F32, tag="pv")
                for jb in range(njb):
                    skip = 128 * max(0, jb - 4 * ch)
                    nc.tensor.matmul(
                        pso[:, skip:512],
                        v_aug[:, jb * 65:(jb + 1) * 65],
                        attn[:, jb * 512 + skip:(jb + 1) * 512],
                        start=(jb == 0), stop=(jb == njb - 1))
                nc.vector.tensor_copy(outT[:, ch * 512:(ch + 1) * 512], pso[:])
                # ---- normalize this chunk's 4 i-blocks
                for jb in range(4 * ch, 4 * ch + 4):
                    tr = tr_ps.tile([128, 65], F32, tag="tr")
                    nc.tensor.transpose(tr[:], outT[:, jb * 128:(jb + 1) * 128],
                                        ident[0:65, 0:65])
                    rec = recp.tile([128, 1], F32, tag="rec")
                    nc.vector.reciprocal(rec[:], tr[:, 64:65])
                    nc.vector.tensor_scalar_mul(stage[:, jb * 64:(jb + 1) * 64],
                                                tr[:, 0:64], rec[:])
            nc.sync.dma_start(out_d[b].rearrange("(n p) h -> p n h", p=128),
                              stage[:].rearrange("p (n h) -> p n h", n=NIB))

    nc.compile()
    return nc


_NC = None


def kernel(x, Wk, Wq, Wv):
    global _NC, LAST_RESULT
    if _NC is None:
        _NC = _build()

    xt = _round_fp32r(
        np.ascontiguousarray(
            x.reshape(NCORES, BPC, S, E).transpose(0, 1, 3, 2)))
    wqk = _round_fp32r(np.concatenate([Wq.T, Wk.T], axis=1))
    wkv = _round_fp32r(np.concatenate([Wk.T, Wv.T], axis=1))
    ident = np.eye(128, dtype=np.float32)
    idhi = np.zeros((128, 64), dtype=np.float32)
    idhi[64:128, :] = np.eye(64, dtype=np.float32)
    tri = np.triu(np.ones((128, 128), dtype=np.float32))

    in_maps = [
        {"xt": np.ascontiguousarray(xt[c]), "wqk": wqk, "wkv": wkv,
         "ident": ident, "idhi": idhi, "tri": tri}
        for c in range(NCORES)
    ]
    trace = os.environ.get("KERNEL_TRACE") == "1"
    try:
        res = bass_utils.run_bass_kernel_spmd(
            _NC, in_maps, core_ids=list(range(NCORES)), trace=trace)
    except (ImportError, ModuleNotFoundError):
        res = bass_utils.run_bass_kernel_spmd(
            _NC, in_maps, core_ids=list(range(NCORES)), trace=False)
    LAST_RESULT = res
    out = np.concatenate([res.results[c]["out"] for c in range(NCORES)], axis=0)
    return out.astype(np.float32)
